# revision 10
# baseline (speedup 1.0000x reference)
"""EnhancedSparseAttention Trainium2 kernel (8 NeuronCores, query-sharded).

Each core computes full 8-head masked attention + out_proj + residual +
LayerNorm for its 512-query slice (rows 8i..8i+8 of the 64x64 grid); the
host concatenates the 8 slices. No collectives.

Per core:
  - k/vT projections from full x; q projection from the core's x slice.
  - scores computed transposed  sT[m, n] = sum_d k[d, m] q[d, n], with two
    heads packed into PE row-groups (K=32 each).
  - mask folded into PSUM by accumulating I128 @ moffT (fp8 0 / -48) onto
    the scores, so exp() of masked entries is ~1e-21.
  - one ACT Exp per [128, 2x512] PSUM pair -> probsT in SBUF.
  - PV: outT[d, n] accumulated over 32 m-chunks; two heads share one PSUM
    bank via col-tile offsets 0 / 64; vT has a ones column so row 32 is
    the softmax denominator.
  - normalize via DVE reciprocal + partition-broadcast multiply, + bv.
  - out_proj + bo + residual fused (scalar_tensor_tensor); LayerNorm
    channel sums via ones-vector matmuls.
"""

import sys

for _p in ("/opt/trn_rl_repo", "/opt/trn_rl_repo/concourse"):
    if _p not in sys.path:
        sys.path.insert(0, _p)

from contextlib import ExitStack

import ml_dtypes
import numpy as np

import concourse.bass as bass  # noqa: F401
import concourse.mybir as mybir
import concourse.tile as tile
from concourse import bacc
from concourse.bass_utils import run_bass_kernel_spmd

F32 = mybir.dt.float32
F32R = mybir.dt.float32r
FP8 = mybir.dt.float8e4
AF = mybir.ActivationFunctionType
ALU = mybir.AluOpType

HEADS = 8
C = 256
HD = 32
N = 4096
NS = 512          # queries per core
NCORES = 8
MC = 32           # m-chunks of 128
SCALE = HD ** -0.5
LN_EPS = 1e-5
MOFF_VAL = -48.0  # exact in fp8e4m3; exp(-48) ~ 1.4e-21

# smalls column layout: per-partition vectors, [128, 12]
S_BQ, S_BK, S_BV, S_BO, S_GAMMA, S_BETA = 0, 2, 4, 6, 8, 10

_BUILD_CACHE: dict = {}


def build(debug: bool = False):
    nc = bacc.Bacc()

    x_d = nc.dram_tensor("x", [2, 128, N], F32R, kind="ExternalInput")
    xq_d = nc.dram_tensor("xq", [2, 128, NS], F32R, kind="ExternalInput")
    wqt_d = nc.dram_tensor("wqt", [2, 2, 128, 128], F32R, kind="ExternalInput")
    wkt_d = nc.dram_tensor("wkt", [2, 2, 128, 128], F32R, kind="ExternalInput")
    wvt_d = nc.dram_tensor("wvt", [2, 128, C], F32R, kind="ExternalInput")
    wot_d = nc.dram_tensor("wot", [2, 128, 2, 128], F32R, kind="ExternalInput")
    smalls_d = nc.dram_tensor("smalls", [128, 12], F32, kind="ExternalInput")
    ident_d = nc.dram_tensor("ident", [128, 128], FP8, kind="ExternalInput")
    moff_d = nc.dram_tensor("moff", [HEADS, MC, 128, NS], FP8, kind="ExternalInput")
    out_d = nc.dram_tensor("out", [2, 128, NS], F32, kind="ExternalOutput")

    dbg = {}
    if debug:
        dbg["q"] = nc.dram_tensor("dbg_q", [2, 128, NS], F32R, kind="ExternalOutput")
        dbg["k"] = nc.dram_tensor("dbg_k", [2, 128, N], F32R, kind="ExternalOutput")
        dbg["vt"] = nc.dram_tensor("dbg_vt", [128, MC, HEADS, 33], F32R, kind="ExternalOutput")
        dbg["probs"] = nc.dram_tensor("dbg_probs", [128, 1024], F32R, kind="ExternalOutput")
        dbg["attn"] = nc.dram_tensor("dbg_attn", [2, 128, NS], F32R, kind="ExternalOutput")
        dbg["z"] = nc.dram_tensor("dbg_z", [2, 128, NS], F32R, kind="ExternalOutput")

    with tile.TileContext(nc) as tc, ExitStack() as ctx:
        const_p = ctx.enter_context(tc.tile_pool(name="const", bufs=1))
        big_p = ctx.enter_context(tc.tile_pool(name="big", bufs=1))
        moff_p = ctx.enter_context(tc.tile_pool(name="moff", bufs=2))
        probs_p = ctx.enter_context(tc.tile_pool(name="probs", bufs=3))
        row_p = ctx.enter_context(tc.tile_pool(name="rows", bufs=4))
        ps_s = ctx.enter_context(tc.tile_pool(name="ps_s", bufs=2, space="PSUM"))
        ps_o = ctx.enter_context(tc.tile_pool(name="ps_o", bufs=1, space="PSUM"))
        ps_w = ctx.enter_context(tc.tile_pool(name="ps_w", bufs=2, space="PSUM"))
        dram_p = ctx.enter_context(tc.tile_pool(name="dram", bufs=2, space="DRAM"))

        # ---------------- constants / inputs ----------------
        x_sb = big_p.tile([128, 2, N], F32R)
        nc.sync.dma_start(out=x_sb[:], in_=x_d[:, :, :].rearrange("a p n -> p a n"))
        xq_sb = big_p.tile([128, 2, NS], F32R)
        nc.sync.dma_start(out=xq_sb[:], in_=xq_d[:, :, :].rearrange("a p n -> p a n"))

        wqt_sb = const_p.tile([128, 2, 2, 128], F32R)
        nc.sync.dma_start(out=wqt_sb[:], in_=wqt_d[:, :, :, :].rearrange("g a p m -> p g a m"))
        wkt_sb = const_p.tile([128, 2, 2, 128], F32R)
        nc.sync.dma_start(out=wkt_sb[:], in_=wkt_d[:, :, :, :].rearrange("g a p m -> p g a m"))
        wvt_sb = const_p.tile([128, 2, C], F32R)
        nc.sync.dma_start(out=wvt_sb[:], in_=wvt_d[:, :, :].rearrange("a p m -> p a m"))
        wot_sb = const_p.tile([128, 2, 2, 128], F32R)
        nc.sync.dma_start(out=wot_sb[:], in_=wot_d[:, :, :, :].rearrange("a p o m -> p a o m"))
        smalls_sb = const_p.tile([128, 12], F32)
        nc.sync.dma_start(out=smalls_sb[:], in_=smalls_d[:, :])
        ident_sb = const_p.tile([128, 128], FP8)
        nc.sync.dma_start(out=ident_sb[:], in_=ident_d[:, :])

        # ---------------- q projection: [128, 2, NS] ----------------
        q_sb = big_p.tile([128, 2, NS], F32R)
        for g in range(2):
            pq = ps_w.tile([128, NS], F32, tag="w")
            for a in range(2):
                nc.tensor.matmul(
                    pq[:], wqt_sb[:, g, a, :], xq_sb[:, a, :],
                    start=(a == 0), stop=(a == 1),
                )
            nc.vector.tensor_scalar_add(
                q_sb[:, g, :], pq[:], smalls_sb[:, S_BQ + g : S_BQ + g + 1]
            )

        # ---------------- k projection: [128, 2, N] ----------------
        k_sb = big_p.tile([128, 2, N], F32R)
        for o in range(2):
            for t in range(8):
                pk = ps_w.tile([128, NS], F32, tag="w")
                for a in range(2):
                    nc.tensor.matmul(
                        pk[:], wkt_sb[:, o, a, :], x_sb[:, a, t * 512 : (t + 1) * 512],
                        start=(a == 0), stop=(a == 1),
                    )
                nc.vector.tensor_scalar_add(
                    k_sb[:, o, t * 512 : (t + 1) * 512], pk[:],
                    smalls_sb[:, S_BK + o : S_BK + o + 1],
                )

        # ---------------- vT: [128, MC, HEADS, 33] ----------------
        vt_sb = big_p.tile([128, MC, HEADS, 33], F32R)
        nc.vector.memset(vt_sb[:, :, :, 32:33].bitcast(F32), 1.0)
        for mc in range(MC):
            pv = ps_w.tile([128, NS], F32, tag="w")
            for a in range(2):
                nc.tensor.matmul(
                    pv[:, 0:C], x_sb[:, a, mc * 128 : (mc + 1) * 128], wvt_sb[:, a, :],
                    start=(a == 0), stop=(a == 1),
                )
            nc.vector.tensor_copy(
                vt_sb[:, mc, :, 0:32], pv[:, 0:C].rearrange("p (h d) -> p h d", h=HEADS)
            )
        if debug:
            nc.sync.dma_start(out=dbg["vt"][:, :, :, :], in_=vt_sb[:])

        # ---------------- main attention loop ----------------
        attn_sb = big_p.tile([128, 2, NS], F32R)
        for pair in range(4):
            g = pair // 2
            sub = pair % 2          # partition offset 64*sub within chunk g
            h0 = 2 * pair           # heads h0, h0+1
            po0 = ps_o.tile([33, NS], F32, tag="o0")
            po1 = ps_o.tile([33, NS], F32, tag="o1")
            pos = (po0, po1)
            for half in range(2):
                mt = moff_p.tile([128, 2, 16, NS], FP8, tag="m")
                for b in range(2):
                    nc.sync.dma_start(
                        out=mt[:, b, :, :],
                        in_=moff_d[h0 + b, half * 16 : (half + 1) * 16, :, :].rearrange(
                            "t p n -> p t n"
                        ),
                    )
                for t in range(16):
                    mc = half * 16 + t
                    pscore = ps_s.tile([128, 2, 512], F32, tag="s")
                    for b in range(2):
                        bp = 64 * sub + 32 * b
                        nc.tensor.matmul(
                            pscore[:, b, :],
                            k_sb[bp : bp + 32, g, mc * 128 : (mc + 1) * 128],
                            q_sb[bp : bp + 32, g, :],
                            start=True, stop=False,
                            tile_position=(bp, 0),
                        )
                    for b in range(2):
                        nc.tensor.matmul(
                            pscore[:, b, :], ident_sb[:], mt[:, b, t, :],
                            start=False, stop=True,
                        )
                    probs = probs_p.tile([128, 2, 512], F32R, tag="p")
                    nc.scalar.activation(probs[:], pscore[:], AF.Exp)
                    if debug and pair == 0 and mc == 0:
                        nc.sync.dma_start(
                            out=dbg["probs"][:, :],
                            in_=probs[:].rearrange("p a n -> p (a n)"),
                        )
                    for b in range(2):
                        nc.tensor.matmul(
                            pos[b][:, :],
                            vt_sb[:, mc, h0 + b, :],
                            probs[:, b, :],
                            start=(mc == 0), stop=(mc == MC - 1),
                        )
            # normalize: attn = po[0:32]/po[32] + bv  (and 64:96 / 96)
            rb = row_p.tile([128, NS], F32, tag="rb")
            for b in range(2):
                recip = row_p.tile([1, NS], F32, tag="r")
                nc.vector.reciprocal(recip[:], pos[b][32:33, :])
                pp = 64 * sub + 32 * b
                rd = dram_p.tile([1, NS], F32, tag="rd")
                nc.sync.dma_start(out=rd[:], in_=recip[:])
                nc.sync.dma_start(
                    out=rb[pp : pp + 32, :],
                    in_=rd[0:1, :].partition_broadcast(32),
                )
                dst = attn_sb[pp : pp + 32, g, :]
                nc.vector.tensor_tensor(
                    dst, pos[b][0:32, :], rb[pp : pp + 32, :], ALU.mult,
                )
                nc.vector.tensor_scalar_add(
                    dst, dst, smalls_sb[pp : pp + 32, S_BV + g : S_BV + g + 1]
                )
        if debug:
            nc.sync.dma_start(out=dbg["attn"][:, :, :].rearrange("a p n -> p a n"), in_=attn_sb[:])
            nc.sync.dma_start(out=dbg["q"][:, :, :].rearrange("a p n -> p a n"), in_=q_sb[:])
            nc.sync.dma_start(out=dbg["k"][:, :, :].rearrange("a p n -> p a n"), in_=k_sb[:])

        # ---------------- out_proj + residual ----------------
        z_sb = big_p.tile([128, 2, NS], F32R)
        z2_sb = big_p.tile([128, 2, NS], F32R)
        for o in range(2):
            pz = ps_w.tile([128, NS], F32, tag="w")
            for a in range(2):
                nc.tensor.matmul(
                    pz[:], wot_sb[:, a, o, :], attn_sb[:, a, :],
                    start=(a == 0), stop=(a == 1),
                )
            nc.vector.scalar_tensor_tensor(
                out=z_sb[:, o, :], in0=pz[:],
                scalar=smalls_sb[:, S_BO + o : S_BO + o + 1],
                in1=xq_sb[:, o, :],
                op0=ALU.add, op1=ALU.add,
            )
            nc.scalar.square(z2_sb[:, o, :], z_sb[:, o, :])
        if debug:
            nc.sync.dma_start(out=dbg["z"][:, :, :].rearrange("a p n -> p a n"), in_=z_sb[:])

        # ---------------- LayerNorm over channels ----------------
        ones_sb = const_p.tile([128, 1], F32R)
        nc.vector.memset(ones_sb[:].bitcast(F32), 1.0)
        psum_sum = ps_s.tile([1, NS], F32, tag="s")
        psum_sq = ps_s.tile([1, NS], F32, tag="s")
        for a in range(2):
            nc.tensor.matmul(psum_sum[:], ones_sb[:], z_sb[:, a, :], start=(a == 0), stop=(a == 1))
        for a in range(2):
            nc.tensor.matmul(psum_sq[:], ones_sb[:], z2_sb[:, a, :], start=(a == 0), stop=(a == 1))

        mu = row_p.tile([1, NS], F32, tag="r")
        nc.vector.tensor_scalar_mul(mu[:], psum_sum[:], 1.0 / C)
        msq = row_p.tile([1, NS], F32, tag="r")
        nc.vector.tensor_scalar_mul(msq[:], psum_sq[:], 1.0 / C)
        var = row_p.tile([1, NS], F32, tag="r")
        nc.vector.tensor_tensor(var[:], mu[:], mu[:], ALU.mult)
        nc.vector.tensor_tensor(var[:], msq[:], var[:], ALU.subtract)
        eps_sb = const_p.tile([1, 1], F32)
        nc.vector.memset(eps_sb[:], LN_EPS)
        std = row_p.tile([1, NS], F32, tag="r")
        nc.scalar.activation(std[:], var[:], AF.Sqrt, bias=eps_sb[:])
        rs = row_p.tile([1, NS], F32, tag="r")
        nc.vector.reciprocal(rs[:], std[:])

        mu_d = dram_p.tile([1, NS], F32, tag="rd")
        nc.sync.dma_start(out=mu_d[:], in_=mu[:])
        rs_d = dram_p.tile([1, NS], F32, tag="rd")
        nc.sync.dma_start(out=rs_d[:], in_=rs[:])
        mu_b = row_p.tile([128, NS], F32, tag="rb")
        nc.sync.dma_start(out=mu_b[:], in_=mu_d[0:1, :].partition_broadcast(128))
        rs_b = row_p.tile([128, NS], F32, tag="rb")
        nc.sync.dma_start(out=rs_b[:], in_=rs_d[0:1, :].partition_broadcast(128))
        out_sb = big_p.tile([128, 2, NS], F32)
        for a in range(2):
            nc.vector.tensor_tensor(
                out_sb[:, a, :], z_sb[:, a, :], mu_b[:], ALU.subtract,
            )
            nc.vector.tensor_tensor(
                out_sb[:, a, :], out_sb[:, a, :], rs_b[:], ALU.mult,
            )
            nc.vector.tensor_scalar(
                out_sb[:, a, :], out_sb[:, a, :],
                smalls_sb[:, S_GAMMA + a : S_GAMMA + a + 1],
                smalls_sb[:, S_BETA + a : S_BETA + a + 1],
                op0=ALU.mult, op1=ALU.add,
            )
        nc.sync.dma_start(out=out_d[:, :, :].rearrange("a p n -> p a n"), in_=out_sb[:])

    nc.compile()
    return nc, dbg


def host_prep(x, mask, Wq, bq, Wk, bk, Wv, bv, Wo, bo, gamma, beta):
    """Build the 8 per-core input maps."""
    x2d = np.ascontiguousarray(np.asarray(x, np.float32).reshape(C, N))
    xr = np.ascontiguousarray(x2d.reshape(2, 128, N))

    def wt_chunks(W, scale=1.0):
        # [g out-chunk, a c-chunk, c-in-chunk (partition), m out col]
        out = np.empty((2, 2, 128, 128), np.float32)
        for g in range(2):
            for a in range(2):
                out[g, a] = (
                    scale * np.asarray(W, np.float32)[128 * g : 128 * (g + 1), 128 * a : 128 * (a + 1)]
                ).T
        return np.ascontiguousarray(out)

    wqt = wt_chunks(Wq, SCALE)
    wkt = wt_chunks(Wk)
    wvt = np.ascontiguousarray(np.asarray(Wv, np.float32).T.reshape(2, 128, C))
    wot = np.ascontiguousarray(np.asarray(Wo, np.float32).T.reshape(2, 128, 2, 128))

    smalls = np.zeros((128, 12), np.float32)
    bq_s = SCALE * np.asarray(bq, np.float32)
    for g in range(2):
        sl = slice(128 * g, 128 * (g + 1))
        smalls[:, S_BQ + g] = bq_s[sl]
        smalls[:, S_BK + g] = np.asarray(bk, np.float32)[sl]
        smalls[:, S_BV + g] = np.asarray(bv, np.float32)[sl]
        smalls[:, S_BO + g] = np.asarray(bo, np.float32)[sl]
        smalls[:, S_GAMMA + g] = np.asarray(gamma, np.float32)[sl]
        smalls[:, S_BETA + g] = np.asarray(beta, np.float32)[sl]

    moff_byte = np.float32(MOFF_VAL).astype(ml_dtypes.float8_e4m3).view(np.uint8)
    ident = np.ascontiguousarray(np.eye(128, dtype=np.float32).astype(ml_dtypes.float8_e4m3))

    mask_np = np.asarray(mask[0])  # [H, N, N] bool
    in_maps = []
    for i in range(NCORES):
        ns = slice(NS * i, NS * (i + 1))
        mT = np.ascontiguousarray(mask_np[:, ns, :].transpose(0, 2, 1))  # [H, 4096, 512]
        moff_u8 = np.where(mT, np.uint8(0), moff_byte)
        moff = moff_u8.view(ml_dtypes.float8_e4m3).reshape(HEADS, MC, 128, NS)
        xq = np.ascontiguousarray(x2d[:, ns].reshape(2, 128, NS))
        in_maps.append(
            {
                "x": xr, "xq": xq,
                "wqt": wqt, "wkt": wkt, "wvt": wvt, "wot": wot,
                "smalls": smalls, "ident": ident, "moff": moff,
            }
        )
    return in_maps


def kernel(**inputs):
    if "nc" not in _BUILD_CACHE:
        _BUILD_CACHE["nc"] = build(debug=False)
    nc, _ = _BUILD_CACHE["nc"]
    in_maps = host_prep(**inputs)
    res = run_bass_kernel_spmd(nc, in_maps, core_ids=list(range(NCORES)))
    full = np.empty((1, C, 64, 64), np.float32)
    for i in range(NCORES):
        o = res.results[i]["out"].reshape(C, NS)
        full[0, :, 8 * i : 8 * (i + 1), :] = o.reshape(C, 8, 64)
    return full
